# revision 9
# baseline (speedup 1.0000x reference)
"""Adaptive-GCN Trainium2 kernel: 8-core data-parallel Bass implementation.

Sharding: graphs/nodes are split contiguously across 8 cores (125 graphs /
12500 nodes per core).  Edges are assigned to the core owning their
destination node; the v[edge_src] gather reads per-window deduplicated
v-row tables staged in each core's DRAM, so execution needs no cross-core
communication.

Edges are re-packed on the host into 128-edge tiles grouped by 512-node
destination windows.  Tile structure (counts, node-base offsets) is
synchronized across cores so one SPMD program fits all cores; per-core
variability lives in DMA'd data (edge features, gather indices, 0/1
scatter matrices S).

  ke  = e_tile^T @ [K_w; K_b]      (PE; edge features are the stationary side)
  ve  = leakyrelu(ke * v[src])     (DVE multiply, ACT Lrelu)
  sve = sum_tiles  ve_tile^T @ S   (PE, accumulated into a PSUM node-window)

Node/graph phases run feature-major ([128 features x nodes]) with bf16
matmul operands and f32 PSUM accumulation.
"""
import sys

sys.path.insert(0, "/opt/trn_rl_repo")

import numpy as np
import ml_dtypes

from concourse import bacc, mybir, tile
import concourse.tile_utils as _tu

_tu.max_sbuf_usage = 200 * 1024  # stale 192K cap; cayman has 208K usable

dt = mybir.dt
Alu = mybir.AluOpType
Act = mybir.ActivationFunctionType
Axis = mybir.AxisListType
BF16 = ml_dtypes.bfloat16

# ---------------- problem constants (must match reference.py) --------------
NCORES = 8
N, NE, G = 100_000, 800_000, 1_000
VD, ED, H, KH = 128, 64, 128, 4
NC = N // NCORES          # 12500 nodes per core
GC = G // NCORES          # 125 graphs per core
RG = N // G               # 100 nodes per graph
WIN = 512                 # scatter window (one PSUM bank of f32)
SW = 64                   # S-matrix node span per edge tile
SUB = 4                   # edge tiles per elementwise batch (512 edges)
SEGT = 32                 # edge tiles per gather call / edge-stream buffer
CHUNK = 500               # node chunk for phases 2/3 (5 graphs)
NCH = NC // CHUNK         # 25 chunks
GPC = CHUNK // RG         # graphs per chunk (5)

WIN_STARTS = list(range(0, NC, WIN))
WIN_SIZES = [min(WIN, NC - s) for s in WIN_STARTS]
NWIN = len(WIN_STARTS)


# ===========================================================================
# Host-side sharding / packing
# ===========================================================================

def _pack_edges(edge_src, edge_dst):
    """Assign edges to cores by dst; pack into synchronized 128-edge tiles."""
    edge_dst = np.asarray(edge_dst).astype(np.int64)
    core_of = edge_dst // NC

    per_cw = [[None] * NWIN for _ in range(NCORES)]
    for c in range(NCORES):
        m = np.nonzero(core_of == c)[0]
        d = edge_dst[m] - c * NC
        order = np.argsort(d, kind="stable")
        m, d = m[order], d[order]
        for wi in range(NWIN):
            lo = np.searchsorted(d, WIN_STARTS[wi], side="left")
            hi = np.searchsorted(d, WIN_STARTS[wi] + WIN_SIZES[wi], side="left")
            per_cw[c][wi] = (m[lo:hi], d[lo:hi] - WIN_STARTS[wi])

    tiles_base, tiles_window, win_tile0 = [], [], []
    slot_eid = [[] for _ in range(NCORES)]
    for wi in range(NWIN):
        wsz = WIN_SIZES[wi]
        ptr = [0] * NCORES
        arrs = [per_cw[c][wi] for c in range(NCORES)]
        win_tile0.append(len(tiles_base))
        while True:
            active = [c for c in range(NCORES) if ptr[c] < len(arrs[c][0])]
            if not active:
                break
            base = min(int(arrs[c][1][ptr[c]]) for c in active)
            base = min(base, max(0, wsz - SW))
            for c in range(NCORES):
                eids, d = arrs[c]
                j0 = ptr[c]
                hi = np.searchsorted(d, base + SW, side="left")
                take = min(hi - j0, 128)
                sl = np.full(128, -1, dtype=np.int64)
                sl[:take] = eids[j0:j0 + take]
                slot_eid[c].append(sl)
                ptr[c] = j0 + take
            tiles_base.append(base)
            tiles_window.append(wi)
        if win_tile0[-1] == len(tiles_base):
            for c in range(NCORES):
                slot_eid[c].append(np.full(128, -1, dtype=np.int64))
            tiles_base.append(0)
            tiles_window.append(wi)

    T_TOT = len(tiles_base)
    T_W = [0] * NWIN
    for wi in tiles_window:
        T_W[wi] += 1
    meta = dict(T_TOT=T_TOT, T_W=T_W, WIN_TILE0=win_tile0,
                BASE=np.array(tiles_base), TWIN=np.array(tiles_window))
    # gather segments: SEGT tiles each; record base window + window span
    nseg = (T_TOT + SEGT - 1) // SEGT
    seg_w0, seg_wspan = [], []
    for si in range(nseg):
        t0, t1 = si * SEGT, min((si + 1) * SEGT, T_TOT)
        ws = [tiles_window[t] for t in range(t0, t1)]
        seg_w0.append(min(ws))
        seg_wspan.append(max(ws) - min(ws) + 1)
    meta["NSEG"] = nseg
    meta["SEG_W0"] = seg_w0
    meta["SEG_WSPAN"] = seg_wspan
    return meta, [np.stack(s) for s in slot_eid]


def _compute_dmax(meta, slot_eid, edge_src):
    DMAX = 1
    for c in range(NCORES):
        for wi in range(NWIN):
            t0, tw = meta["WIN_TILE0"][wi], meta["T_W"][wi]
            sl = slot_eid[c][t0:t0 + tw]
            va = sl >= 0
            if va.any():
                DMAX = max(DMAX, len(np.unique(edge_src[sl[va]])))
    return DMAX


def _stage_core(c, meta, slots, inputs, DMAX):
    v = np.asarray(inputs["v"], np.float32)
    e = np.asarray(inputs["e"], np.float32)
    edge_src = np.asarray(inputs["edge_src"]).astype(np.int64)
    edge_dst = np.asarray(inputs["edge_dst"]).astype(np.int64)
    T_TOT, BASE, TWIN = meta["T_TOT"], meta["BASE"], meta["TWIN"]

    valid = slots >= 0                     # [T_TOT, 128]
    eids = np.where(valid, slots, 0)

    e_fm = np.zeros((65, T_TOT * 128), np.float32)
    cols = np.arange(T_TOT)[:, None] * 128 + np.arange(128)[None, :]
    e_fm[:64, cols[valid]] = e[eids[valid]].T
    e_fm[64, :] = 1.0

    S = np.zeros((128, T_TOT * SW), np.float32)
    wstart = np.array(WIN_STARTS)[TWIN]
    d_local = edge_dst[eids] - c * NC - wstart[:, None]
    off = d_local - BASE[:, None]
    tv, sv = np.nonzero(valid)
    offv = off[tv, sv]
    assert (offv >= 0).all() and (offv < SW).all()
    S[sv, tv * SW + offv] = 1.0

    # per-window dedup; per-SEGMENT indices (relative to segment's base window)
    vv = np.zeros((NWIN * DMAX, VD), np.float32)
    upos = np.zeros((T_TOT, 128), np.int64)     # window-local dedup position
    for wi in range(NWIN):
        t0, tw = meta["WIN_TILE0"][wi], meta["T_W"][wi]
        sl = slots[t0:t0 + tw]
        va = sl >= 0
        srcs = edge_src[np.where(va, sl, 0)]
        if va.any():
            u = np.unique(srcs[va])
        else:
            u = np.array([0], np.int64)
        assert len(u) <= DMAX
        vv[wi * DMAX: wi * DMAX + len(u)] = v[u]
        pos = np.searchsorted(u, srcs)
        upos[t0:t0 + tw] = np.where(va, pos, 0)

    widx16 = np.zeros((16, T_TOT * 8), np.int16)
    for si in range(meta["NSEG"]):
        t0, t1 = si * SEGT, min((si + 1) * SEGT, T_TOT)
        w0 = meta["SEG_W0"][si]
        rel = (TWIN[t0:t1] - w0) * DMAX
        vals = upos[t0:t1] + rel[:, None]
        assert vals.max() < 32768
        flat = vals.reshape(-1).astype(np.int16)
        i = np.arange(flat.size)
        widx16[i % 16, t0 * 8 + i // 16] = flat
    widx = np.tile(widx16, (8, 1))

    return dict(e_fm=e_fm.astype(BF16), S=S.astype(BF16),
                vv=vv.astype(BF16), widx=widx)


def _stage_weights(inputs):
    f32 = lambda k: np.asarray(inputs[k], np.float32)
    cols, colmap = [], {}

    def addw(name, arr):
        colmap[name] = sum(a.shape[1] for a in cols)
        cols.append(arr)

    addw("A", f32("A_w"))
    addw("C", f32("C_w"))
    addw("mA", np.concatenate([f32("mA_w")[k] for k in range(KH)], axis=1))
    addw("mB", np.concatenate([f32("mB_w")[k] for k in range(KH)], axis=1))
    addw("mD", np.concatenate([f32("mD_w")[k] for k in range(KH)], axis=1))
    addw("B", np.concatenate([f32("B_w")[128 * k:128 * (k + 1)] for k in range(KH)], axis=1))
    addw("E", np.concatenate([f32("E_w")[:128], f32("E_w")[128:]], axis=1))
    addw("gmA", f32("gm_A_w"))
    addw("gmB", f32("gm_B_w"))
    addw("gmWih", f32("gm_Wih"))
    addw("gmWhh", f32("gm_Whh"))
    addw("gsA", f32("gs_A_w"))
    addw("gsB", f32("gs_B_w"))
    addw("gsWih", f32("gs_Wih"))
    addw("gsWhh", f32("gs_Whh"))
    wpack = np.concatenate(cols, axis=1).astype(BF16)

    bcols, bmap = [], {}

    def addb(name, vec):
        bmap[name] = len(bcols)
        bcols.append(np.asarray(vec, np.float32).reshape(128))

    addb("A_b", f32("A_b"))
    addb("C_b", f32("C_b"))
    addb("E_b", f32("E_b"))
    addb("B_b", f32("B_b") + f32("mD_b").reshape(KH * H) @ f32("B_w"))
    addb("gmz", f32("gm_A_b") + f32("gm_B_b"))
    addb("gm_br", f32("gm_bih")[:128] + f32("gm_bhh")[:128])
    addb("gm_bz", f32("gm_bih")[128:256] + f32("gm_bhh")[128:256])
    addb("gm_bihn", f32("gm_bih")[256:])
    addb("gm_bhhn", f32("gm_bhh")[256:])
    addb("gsz", f32("gs_A_b") + f32("gs_B_b"))
    addb("gs_br", f32("gs_bih")[:128] + f32("gs_bhh")[:128])
    addb("gs_bz", f32("gs_bih")[128:256] + f32("gs_bhh")[128:256])
    addb("gs_bihn", f32("gs_bih")[256:])
    addb("gs_bhhn", f32("gs_bhh")[256:])
    for k in range(KH):
        addb(f"mA_b{k}", f32("mA_b")[k])
    for k in range(KH):
        addb(f"mB_b{k}", f32("mB_b")[k])
    for k in range(KH):
        addb(f"mC_{k}", f32("mC_w")[k])
    biases = np.stack(bcols, axis=1)

    kwb = np.concatenate([f32("K_w"), f32("K_b")[None, :]], axis=0).astype(BF16)
    return wpack, colmap, biases, bmap, kwb


# ===========================================================================
# Program builder
# ===========================================================================

def build_program(meta, DMAX):
    T_TOT, T_W, WIN_TILE0 = meta["T_TOT"], meta["T_W"], meta["WIN_TILE0"]
    BASE, TWIN = meta["BASE"], meta["TWIN"]
    NSEG, SEG_W0, SEG_WSPAN = meta["NSEG"], meta["SEG_W0"], meta["SEG_WSPAN"]
    COLMAP, BMAP, NB = meta["COLMAP"], meta["BMAP"], meta["NB"]
    WPACK_COLS = meta["WPACK_COLS"]

    nc = bacc.Bacc("TRN2")
    d_vv = nc.dram_tensor("vv", [NWIN * DMAX, VD], dt.bfloat16, kind="ExternalInput")
    d_widx = nc.dram_tensor("widx", [128, T_TOT * 8], dt.int16, kind="ExternalInput")
    d_efm = nc.dram_tensor("e_fm", [65, T_TOT * 128], dt.bfloat16, kind="ExternalInput")
    d_S = nc.dram_tensor("S", [128, T_TOT * SW], dt.bfloat16, kind="ExternalInput")
    d_vfm = nc.dram_tensor("v_fm", [128, NC], dt.bfloat16, kind="ExternalInput")
    d_sfm = nc.dram_tensor("s_fm", [128, GC], dt.bfloat16, kind="ExternalInput")
    d_w = nc.dram_tensor("wpack", [128, WPACK_COLS], dt.bfloat16, kind="ExternalInput")
    d_kwb = nc.dram_tensor("kwb", [65, 128], dt.bfloat16, kind="ExternalInput")
    d_bias = nc.dram_tensor("biases", [128, NB], dt.float32, kind="ExternalInput")
    d_oh = nc.dram_tensor("onehot", [4, KH * 128], dt.bfloat16, kind="ExternalInput")
    d_outv = nc.dram_tensor("out_v", [128, NC], dt.float32, kind="ExternalOutput")
    d_outs = nc.dram_tensor("out_s", [128, GC], dt.float32, kind="ExternalOutput")

    with tile.TileContext(nc) as tc:
        with (
            tc.tile_pool(name="const", bufs=1) as cpool,
            tc.tile_pool(name="pers", bufs=1) as ppool,
            tc.tile_pool(name="ewin", bufs=2) as ewin,
            tc.tile_pool(name="chk", bufs=2) as chk,
            tc.tile_pool(name="dn", bufs=2) as dnp,
            tc.tile_pool(name="psw", bufs=2, space="PSUM") as psw,
            tc.tile_pool(name="pske", bufs=2, space="PSUM") as pske,
            tc.tile_pool(name="pssm", bufs=3, space="PSUM") as pssm,
            tc.tile_pool(name="psa", bufs=1, space="PSUM") as psa,
        ):
            # ---------------- constants
            w_sb = cpool.tile([128, WPACK_COLS], dt.bfloat16)
            nc.sync.dma_start(w_sb[:], d_w[:])
            kwb_sb = cpool.tile([65, 128], dt.bfloat16)
            nc.sync.dma_start(kwb_sb[:], d_kwb[:])
            bias_sb = cpool.tile([128, NB], dt.float32)
            nc.sync.dma_start(bias_sb[:], d_bias[:])
            vfm_sb = ppool.tile([128, NC], dt.bfloat16)
            nc.sync.dma_start(vfm_sb[:], d_vfm[:])
            sfm_sb = cpool.tile([128, GC], dt.bfloat16)
            nc.sync.dma_start(sfm_sb[:], d_sfm[:])

            def W(name, j0=0, w=128):
                o = COLMAP[name]
                return w_sb[:, o + j0: o + j0 + w]

            def B(name):
                return bias_sb[:, BMAP[name]:BMAP[name] + 1]

            # ---------------- persistent intermediates
            sve_sb = ppool.tile([128, NC], dt.bfloat16)
            aexp_sb = ppool.tile([4, GC, RG], dt.bfloat16)
            pooled_sb = ppool.tile([128, KH, GC], dt.float32)
            pooled_bf = ppool.tile([128, KH * GC], dt.bfloat16)
            s2m_sb = cpool.tile([128, GC], dt.bfloat16)
            denom_sb = cpool.tile([4, GC], dt.float32)
            rden_sb = cpool.tile([4, GC], dt.bfloat16)

            # s2m = tanh(C^T s + C_b)
            ps = pssm.tile([128, CHUNK], dt.float32, tag="ps500")
            nc.tensor.matmul(ps[:, :GC], W("C"), sfm_sb[:], start=True, stop=True)
            nc.scalar.activation(s2m_sb[:], ps[:, :GC], Act.Tanh, bias=B("C_b"))

            # ---------------- P1: edge pipeline (gather segments x windows)
            pw = None
            cur_w = -1
            for si in range(NSEG):
                t0, t1 = si * SEGT, min((si + 1) * SEGT, T_TOT)
                nt = t1 - t0
                e_sb = ewin.tile([65, nt * 128], dt.bfloat16, tag="e_sb")
                nc.sync.dma_start(e_sb[:], d_efm[:, t0 * 128: t1 * 128])
                s_sb = ewin.tile([128, nt * SW], dt.bfloat16, tag="s_sb")
                nc.sync.dma_start(s_sb[:], d_S[:, t0 * SW: t1 * SW])
                wix = ewin.tile([128, nt * 8], dt.int16, tag="wix")
                nc.sync.dma_start(wix[:], d_widx[:, t0 * 8: t1 * 8])
                vsrc = ewin.tile([128, nt, 128], dt.bfloat16, tag="vsrc")
                w0 = SEG_W0[si]
                rows = min(SEG_WSPAN[si] * DMAX, NWIN * DMAX - w0 * DMAX)
                for g0 in range(0, nt, 8):
                    g1 = min(g0 + 8, nt)
                    nc.gpsimd.dma_gather(
                        vsrc[:, g0:g1, :], d_vv[w0 * DMAX: w0 * DMAX + rows, :],
                        wix[:, g0 * 8: g1 * 8], (g1 - g0) * 128, (g1 - g0) * 128,
                        VD, queue_num=0)
                ve = ewin.tile([128, nt, 128], dt.bfloat16, tag="ve")
                for s0 in range(0, nt, SUB):
                    s1 = min(s0 + SUB, nt)
                    kep = pske.tile([128, SUB * 128], dt.float32, tag="kep")
                    for t in range(s0, s1):
                        nc.tensor.matmul(
                            kep[:, (t - s0) * 128: (t - s0 + 1) * 128],
                            e_sb[:, t * 128: (t + 1) * 128],
                            kwb_sb[:], start=True, stop=True)
                    ncols = (s1 - s0) * 128
                    vep = ewin.tile([128, SUB * 128], dt.bfloat16, tag="vep")
                    nc.vector.tensor_tensor(
                        vep[:, :ncols], kep[:, :ncols],
                        vsrc[:, s0:s1, :].rearrange("p a b -> p (a b)"),
                        op=Alu.mult)
                    nc.scalar.activation(
                        ve[:, s0:s1, :].rearrange("p a b -> p (a b)"),
                        vep[:, :ncols], Act.Prelu, alpha=0.1)
                # scatter-accumulate, handling window boundaries
                for t in range(t0, t1):
                    wi = int(TWIN[t])
                    if wi != cur_w:
                        if cur_w >= 0:
                            nc.vector.tensor_copy(
                                sve_sb[:, WIN_STARTS[cur_w]:
                                       WIN_STARTS[cur_w] + WIN_SIZES[cur_w]],
                                pw[:, :WIN_SIZES[cur_w]])
                        pw = psw.tile([128, WIN], dt.float32, tag="pw")
                        nc.vector.memset(pw[:], 0.0)
                        cur_w = wi
                    b = int(BASE[t])
                    nc.tensor.matmul(
                        pw[:, b: b + SW], ve[:, t - t0, :],
                        s_sb[:, (t - t0) * SW: (t - t0 + 1) * SW],
                        start=False, stop=(t == T_TOT - 1 or TWIN[t + 1] != wi),
                        skip_group_check=True)
            nc.vector.tensor_copy(
                sve_sb[:, WIN_STARTS[cur_w]: WIN_STARTS[cur_w] + WIN_SIZES[cur_w]],
                pw[:, :WIN_SIZES[cur_w]])

            # ---------------- P2 prologue: d_super, Wmat (padded per head)
            wmatp = []
            for k in range(KH):
                dsp = pssm.tile([128, CHUNK], dt.float32, tag="ps500")
                nc.tensor.matmul(dsp[:, :GC], W("mB", k * 128), sfm_sb[:],
                                 start=True, stop=True)
                ds_k = cpool.tile([128, GC], dt.bfloat16, tag=f"ds{k}")
                nc.scalar.activation(ds_k[:], dsp[:, :GC], Act.Tanh, bias=B(f"mB_b{k}"))
                wp = cpool.tile([128, GC, KH], dt.bfloat16, tag=f"wmatp{k}")
                nc.vector.memset(wp[:], 0.0)
                nc.vector.tensor_scalar(wp[:, :, k], ds_k[:], B(f"mC_{k}"), None,
                                        op0=Alu.mult)
                wmatp.append(wp)

            # ---------------- P2: attention
            for ci in range(NCH):
                nsl = slice(ci * CHUNK, (ci + 1) * CHUNK)
                dns = []
                for k in range(KH):
                    dnps = pssm.tile([128, CHUNK], dt.float32, tag="ps500")
                    nc.tensor.matmul(dnps[:], W("mA", k * 128), vfm_sb[:, nsl],
                                     start=True, stop=True)
                    dn_k = dnp.tile([128, CHUNK], dt.bfloat16, tag=f"dn{k}")
                    nc.scalar.activation(dn_k[:], dnps[:], Act.Tanh,
                                         bias=B(f"mA_b{k}"))
                    dns.append(dn_k)
                aps = psa.tile([4, CHUNK], dt.float32, tag="aps")
                for gl in range(GPC):
                    g = ci * GPC + gl
                    for k in range(KH):
                        nc.tensor.matmul(
                            aps[:, gl * RG: (gl + 1) * RG],
                            wmatp[k][:, g, :],
                            dns[k][:, gl * RG: (gl + 1) * RG],
                            start=(k == 0), stop=(k == KH - 1))
                nc.scalar.activation(
                    aexp_sb[:, ci * GPC:(ci + 1) * GPC, :],
                    aps[:].rearrange("p (a b) -> p a b", a=GPC), Act.Exp)

            nc.vector.tensor_reduce(denom_sb[:], aexp_sb[:], axis=Axis.X, op=Alu.add)
            with nc.allow_low_precision(reason="bf16 recip ok at 2e-2 gate"):
                nc.vector.reciprocal(rden_sb[:], denom_sb[:])

            # one-hot rows for PE partition-broadcast (row k of a 4-row
            # tensor -> all 128 partitions): oh[c, 128k+h] = (c == k)
            oh_sb = cpool.tile([4, KH * 128], dt.bfloat16)
            nc.sync.dma_start(oh_sb[:], d_oh[:])

            for ci in range(NCH):
                nsl = slice(ci * CHUNK, (ci + 1) * CHUNK)
                for k in range(KH):
                    abp = pssm.tile([128, CHUNK], dt.float32, tag="ps500")
                    nc.tensor.matmul(
                        abp[:].rearrange("p (a b) -> p a b", a=GPC),
                        oh_sb[:, k * 128:(k + 1) * 128],
                        aexp_sb[0:4, ci * GPC:(ci + 1) * GPC, :],
                        start=True, stop=True)
                    ab = chk.tile([128, CHUNK], dt.bfloat16, tag="ab")
                    nc.scalar.activation(ab[:], abp[:], Act.Copy)
                    dvp = pssm.tile([128, CHUNK], dt.float32, tag="ps500")
                    nc.tensor.matmul(dvp[:], W("mD", k * 128), vfm_sb[:, nsl],
                                     start=True, stop=True)
                    dvw = chk.tile([128, GPC, RG], dt.bfloat16, tag="dvw")
                    nc.vector.tensor_tensor(
                        dvw[:].rearrange("p a b -> p (a b)"), dvp[:], ab[:],
                        op=Alu.mult)
                    nc.vector.tensor_reduce(
                        pooled_sb[:, k, ci * GPC:(ci + 1) * GPC], dvw[:],
                        axis=Axis.X, op=Alu.add)
            # pooled_bf = pooled_raw * rden (broadcast rden rows via PE)
            rbp = pssm.tile([128, CHUNK], dt.float32, tag="ps500")
            for k in range(KH):
                nc.tensor.matmul(rbp[:, k * GC:(k + 1) * GC],
                                 oh_sb[:, k * 128:(k + 1) * 128],
                                 rden_sb[0:4, :], start=True, stop=True)
            rden_bc = chk.tile([128, KH * GC], dt.bfloat16, tag="rdenbc")
            nc.scalar.activation(rden_bc[:], rbp[:], Act.Copy)
            nc.vector.tensor_tensor(pooled_bf[:],
                                    pooled_sb[:].rearrange("p a b -> p (a b)"),
                                    rden_bc[:], op=Alu.mult)

            # ---------------- P3: message + GRU per node chunk
            for ci in range(NCH):
                nsl = slice(ci * CHUNK, (ci + 1) * CHUNK)
                gsl = slice(ci * GPC, (ci + 1) * GPC)
                s2m_b3 = s2m_sb[:, gsl].unsqueeze(2).to_broadcast([128, GPC, RG])

                def r3(ap):
                    return ap.rearrange("p (a b) -> p a b", a=GPC)

                mp = pssm.tile([128, CHUNK], dt.float32, tag="ps500")
                nc.tensor.matmul(mp[:], W("E", 0), sve_sb[:, nsl], start=True, stop=False)
                nc.tensor.matmul(mp[:], W("E", 128), vfm_sb[:, nsl], start=False, stop=True)
                m2m = chk.tile([128, CHUNK], dt.bfloat16, tag="m2m")
                nc.scalar.activation(m2m[:], mp[:], Act.Prelu, bias=B("E_b"), alpha=0.1)

                zp = pssm.tile([128, CHUNK], dt.float32, tag="ps500")
                nc.tensor.matmul(zp[:], W("gmA"), m2m[:], start=True, stop=False)
                nc.tensor.matmul(zp[:].rearrange("p (a b) -> p a b", a=GPC),
                                 W("gmB"), s2m_b3, start=False, stop=True)
                zt = chk.tile([128, CHUNK], dt.bfloat16, tag="zt")
                nc.scalar.activation(zt[:], zp[:], Act.Sigmoid, bias=B("gmz"))

                t1 = chk.tile([128, CHUNK], dt.bfloat16, tag="t1")
                nc.vector.tensor_tensor(r3(t1[:]), s2m_b3, r3(m2m[:]), op=Alu.subtract)
                t2 = chk.tile([128, CHUNK], dt.bfloat16, tag="t2")
                nc.vector.tensor_tensor(t2[:], zt[:], t1[:], op=Alu.mult)
                hm = chk.tile([128, CHUNK], dt.bfloat16, tag="hm")
                nc.vector.tensor_tensor(hm[:], t2[:], m2m[:], op=Alu.add)

                rp = pssm.tile([128, CHUNK], dt.float32, tag="ps500")
                nc.tensor.matmul(rp[:], W("gmWih", 0), vfm_sb[:, nsl], start=True, stop=False)
                nc.tensor.matmul(rp[:], W("gmWhh", 0), hm[:], start=False, stop=True)
                r = chk.tile([128, CHUNK], dt.bfloat16, tag="r")
                nc.scalar.activation(r[:], rp[:], Act.Sigmoid, bias=B("gm_br"))

                z2p = pssm.tile([128, CHUNK], dt.float32, tag="ps500")
                nc.tensor.matmul(z2p[:], W("gmWih", 128), vfm_sb[:, nsl], start=True, stop=False)
                nc.tensor.matmul(z2p[:], W("gmWhh", 128), hm[:], start=False, stop=True)
                z2 = chk.tile([128, CHUNK], dt.bfloat16, tag="z2")
                nc.scalar.activation(z2[:], z2p[:], Act.Sigmoid, bias=B("gm_bz"))

                innp = pssm.tile([128, CHUNK], dt.float32, tag="ps500")
                nc.tensor.matmul(innp[:], W("gmWih", 256), vfm_sb[:, nsl], start=True, stop=True)
                hnp = pssm.tile([128, CHUNK], dt.float32, tag="ps500")
                nc.tensor.matmul(hnp[:], W("gmWhh", 256), hm[:], start=True, stop=True)
                t3 = chk.tile([128, CHUNK], dt.float32, tag="t3")
                nc.vector.scalar_tensor_tensor(t3[:], hnp[:], B("gm_bhhn"), r[:],
                                               op0=Alu.add, op1=Alu.mult)
                t4 = chk.tile([128, CHUNK], dt.float32, tag="t4")
                nc.vector.tensor_tensor(t4[:], t3[:], innp[:], op=Alu.add)
                n = chk.tile([128, CHUNK], dt.bfloat16, tag="n")
                nc.scalar.activation(n[:], t4[:], Act.Tanh, bias=B("gm_bihn"))
                u1 = chk.tile([128, CHUNK], dt.bfloat16, tag="u1")
                nc.vector.tensor_tensor(u1[:], hm[:], n[:], op=Alu.subtract)
                u2 = chk.tile([128, CHUNK], dt.bfloat16, tag="u2")
                nc.vector.tensor_tensor(u2[:], z2[:], u1[:], op=Alu.mult)
                uo = chk.tile([128, CHUNK], dt.float32, tag="uo")
                nc.vector.tensor_tensor(uo[:], u2[:], n[:], op=Alu.add)
                nc.sync.dma_start(d_outv[:, nsl], uo[:])

            # ---------------- P4: supernode side (125 graphs at once)
            def gmm(w1, r1, w2=None, r2=None):
                p = pssm.tile([128, CHUNK], dt.float32, tag="ps500")
                nc.tensor.matmul(p[:, :GC], w1, r1, start=True, stop=(w2 is None))
                if w2 is not None:
                    nc.tensor.matmul(p[:, :GC], w2, r2, start=False, stop=True)
                return p

            def act(p, func, bias, tag):
                o = chk.tile([128, GC], dt.bfloat16, tag=tag)
                nc.scalar.activation(o[:], p[:, :GC], func, bias=B(bias))
                return o

            s2s = act(gmm(W("A"), sfm_sb[:]), Act.Tanh, "A_b", "s2s")
            p = pssm.tile([128, CHUNK], dt.float32, tag="ps500")
            for k in range(KH):
                nc.tensor.matmul(p[:, :GC], W("B", k * 128),
                                 pooled_bf[:, k * GC:(k + 1) * GC],
                                 start=(k == 0), stop=(k == KH - 1))
            m2s = chk.tile([128, GC], dt.bfloat16, tag="m2s")
            nc.scalar.activation(m2s[:], p[:, :GC], Act.Tanh, bias=B("B_b"))
            zs = act(gmm(W("gsA"), s2s[:], W("gsB"), m2s[:]), Act.Sigmoid, "gsz", "zs")
            st1 = chk.tile([128, GC], dt.bfloat16, tag="st1")
            nc.vector.tensor_tensor(st1[:], m2s[:], s2s[:], op=Alu.subtract)
            st2 = chk.tile([128, GC], dt.bfloat16, tag="st2")
            nc.vector.tensor_tensor(st2[:], zs[:], st1[:], op=Alu.mult)
            hs = chk.tile([128, GC], dt.bfloat16, tag="hs")
            nc.vector.tensor_tensor(hs[:], st2[:], s2s[:], op=Alu.add)
            rs = act(gmm(W("gsWih", 0), sfm_sb[:], W("gsWhh", 0), hs[:]),
                     Act.Sigmoid, "gs_br", "rs")
            z2s = act(gmm(W("gsWih", 128), sfm_sb[:], W("gsWhh", 128), hs[:]),
                      Act.Sigmoid, "gs_bz", "z2s")
            innp = gmm(W("gsWih", 256), sfm_sb[:])
            hnp = gmm(W("gsWhh", 256), hs[:])
            st3 = chk.tile([128, GC], dt.float32, tag="st3")
            nc.vector.scalar_tensor_tensor(st3[:], hnp[:, :GC], B("gs_bhhn"), rs[:],
                                           op0=Alu.add, op1=Alu.mult)
            st4 = chk.tile([128, GC], dt.float32, tag="st4")
            nc.vector.tensor_tensor(st4[:], st3[:], innp[:, :GC], op=Alu.add)
            ns = chk.tile([128, GC], dt.bfloat16, tag="ns")
            nc.scalar.activation(ns[:], st4[:], Act.Tanh, bias=B("gs_bihn"))
            su1 = chk.tile([128, GC], dt.bfloat16, tag="su1")
            nc.vector.tensor_tensor(su1[:], hs[:], ns[:], op=Alu.subtract)
            su2 = chk.tile([128, GC], dt.bfloat16, tag="su2")
            nc.vector.tensor_tensor(su2[:], z2s[:], su1[:], op=Alu.mult)
            suo = chk.tile([128, GC], dt.float32, tag="suo")
            nc.vector.tensor_tensor(suo[:], su2[:], ns[:], op=Alu.add)
            nc.sync.dma_start(d_outs[:], suo[:])

    nc.compile()
    return nc


# ===========================================================================
# Entry point
# ===========================================================================

def prepare(inputs):
    meta, slot_eid = _pack_edges(inputs["edge_src"], inputs["edge_dst"])
    DMAX = _compute_dmax(meta, slot_eid,
                         np.asarray(inputs["edge_src"]).astype(np.int64))
    wpack, colmap, biases, bmap, kwb = _stage_weights(inputs)
    meta["WPACK_COLS"] = wpack.shape[1]
    meta["COLMAP"] = colmap
    meta["BMAP"] = bmap
    meta["NB"] = biases.shape[1]

    v = np.asarray(inputs["v"], np.float32)
    s = np.asarray(inputs["s"], np.float32)
    onehot = np.zeros((4, KH * 128), np.float32)
    for k in range(KH):
        onehot[k, k * 128:(k + 1) * 128] = 1.0
    onehot = onehot.astype(BF16)
    in_maps = []
    for c in range(NCORES):
        st = _stage_core(c, meta, slot_eid[c], inputs, DMAX)
        in_maps.append(dict(
            vv=np.ascontiguousarray(st["vv"]),
            widx=np.ascontiguousarray(st["widx"]),
            e_fm=np.ascontiguousarray(st["e_fm"]),
            S=np.ascontiguousarray(st["S"]),
            v_fm=np.ascontiguousarray(v[c * NC:(c + 1) * NC].T).astype(BF16),
            s_fm=np.ascontiguousarray(s[c * GC:(c + 1) * GC].T).astype(BF16),
            wpack=wpack, kwb=kwb, biases=biases, onehot=onehot,
        ))
    return meta, DMAX, in_maps


def kernel(**inputs):
    meta, DMAX, in_maps = prepare(inputs)
    nc = build_program(meta, DMAX)

    from concourse.bass_utils import run_bass_kernel_spmd
    res = run_bass_kernel_spmd(nc, in_maps, core_ids=list(range(NCORES)))
    upd_v = np.concatenate(
        [np.asarray(res.results[c]["out_v"]).T for c in range(NCORES)], axis=0)
    upd_s = np.concatenate(
        [np.asarray(res.results[c]["out_s"]).T for c in range(NCORES)], axis=0)
    return upd_v.astype(np.float32), upd_s.astype(np.float32)


# revision 10
# speedup vs baseline: 2.1665x; 2.1665x over previous
"""Adaptive-GCN Trainium2 kernel: 8-core data-parallel Bass implementation.

Sharding: graphs/nodes are split contiguously across 8 cores (125 graphs /
12500 nodes per core).  Edges are assigned to the core owning their
destination node; the v[edge_src] gather reads per-window deduplicated
v-row tables staged in each core's DRAM, so execution needs no cross-core
communication.

Edges are re-packed on the host into 128-edge tiles grouped by 512-node
destination windows.  Tile structure (counts, node-base offsets) is
synchronized across cores so one SPMD program fits all cores; per-core
variability lives in DMA'd data (edge features, gather indices, 0/1
scatter matrices S).

  ke  = e_tile^T @ [K_w; K_b]      (PE; edge features are the stationary side)
  ve  = leakyrelu(ke * v[src])     (DVE multiply, ACT Lrelu)
  sve = sum_tiles  ve_tile^T @ S   (PE, accumulated into a PSUM node-window)

Node/graph phases run feature-major ([128 features x nodes]) with bf16
matmul operands and f32 PSUM accumulation.
"""
import sys

sys.path.insert(0, "/opt/trn_rl_repo")

import numpy as np
import ml_dtypes

from concourse import bacc, mybir, tile
import concourse.tile_utils as _tu

_tu.max_sbuf_usage = 200 * 1024  # stale 192K cap; cayman has 208K usable

dt = mybir.dt
Alu = mybir.AluOpType
Act = mybir.ActivationFunctionType
Axis = mybir.AxisListType
BF16 = ml_dtypes.bfloat16

# ---------------- problem constants (must match reference.py) --------------
NCORES = 8
N, NE, G = 100_000, 800_000, 1_000
VD, ED, H, KH = 128, 64, 128, 4
NC = N // NCORES          # 12500 nodes per core
GC = G // NCORES          # 125 graphs per core
RG = N // G               # 100 nodes per graph
WIN = 512                 # scatter window (one PSUM bank of f32)
SW = 64                   # S-matrix node span per edge tile
SUB = 4                   # edge tiles per elementwise batch (512 edges)
SEGT = 32                 # edge tiles per gather call / edge-stream buffer
CHUNK = 500               # node chunk for phases 2/3 (5 graphs)
NCH = NC // CHUNK         # 25 chunks
GPC = CHUNK // RG         # graphs per chunk (5)

WIN_STARTS = list(range(0, NC, WIN))
WIN_SIZES = [min(WIN, NC - s) for s in WIN_STARTS]
NWIN = len(WIN_STARTS)


# ===========================================================================
# Host-side sharding / packing
# ===========================================================================

def _pack_edges(edge_src, edge_dst):
    """Assign edges to cores by dst; pack into synchronized 128-edge tiles."""
    edge_dst = np.asarray(edge_dst).astype(np.int64)
    core_of = edge_dst // NC

    per_cw = [[None] * NWIN for _ in range(NCORES)]
    for c in range(NCORES):
        m = np.nonzero(core_of == c)[0]
        d = edge_dst[m] - c * NC
        order = np.argsort(d, kind="stable")
        m, d = m[order], d[order]
        for wi in range(NWIN):
            lo = np.searchsorted(d, WIN_STARTS[wi], side="left")
            hi = np.searchsorted(d, WIN_STARTS[wi] + WIN_SIZES[wi], side="left")
            per_cw[c][wi] = (m[lo:hi], d[lo:hi] - WIN_STARTS[wi])

    tiles_base, tiles_window, win_tile0 = [], [], []
    slot_eid = [[] for _ in range(NCORES)]
    for wi in range(NWIN):
        wsz = WIN_SIZES[wi]
        ptr = [0] * NCORES
        arrs = [per_cw[c][wi] for c in range(NCORES)]
        win_tile0.append(len(tiles_base))
        while True:
            active = [c for c in range(NCORES) if ptr[c] < len(arrs[c][0])]
            if not active:
                break
            base = min(int(arrs[c][1][ptr[c]]) for c in active)
            base = min(base, max(0, wsz - SW))
            for c in range(NCORES):
                eids, d = arrs[c]
                j0 = ptr[c]
                hi = np.searchsorted(d, base + SW, side="left")
                take = min(hi - j0, 128)
                sl = np.full(128, -1, dtype=np.int64)
                sl[:take] = eids[j0:j0 + take]
                slot_eid[c].append(sl)
                ptr[c] = j0 + take
            tiles_base.append(base)
            tiles_window.append(wi)
        if win_tile0[-1] == len(tiles_base):
            for c in range(NCORES):
                slot_eid[c].append(np.full(128, -1, dtype=np.int64))
            tiles_base.append(0)
            tiles_window.append(wi)

    T_TOT = len(tiles_base)
    T_W = [0] * NWIN
    for wi in tiles_window:
        T_W[wi] += 1
    meta = dict(T_TOT=T_TOT, T_W=T_W, WIN_TILE0=win_tile0,
                BASE=np.array(tiles_base), TWIN=np.array(tiles_window))
    # gather segments: SEGT tiles each; record base window + window span
    nseg = (T_TOT + SEGT - 1) // SEGT
    seg_w0, seg_wspan = [], []
    for si in range(nseg):
        t0, t1 = si * SEGT, min((si + 1) * SEGT, T_TOT)
        ws = [tiles_window[t] for t in range(t0, t1)]
        seg_w0.append(min(ws))
        seg_wspan.append(max(ws) - min(ws) + 1)
    meta["NSEG"] = nseg
    meta["SEG_W0"] = seg_w0
    meta["SEG_WSPAN"] = seg_wspan
    return meta, [np.stack(s) for s in slot_eid]


def _compute_dmax(meta, slot_eid, edge_src):
    DMAX = 1
    for c in range(NCORES):
        for wi in range(NWIN):
            t0, tw = meta["WIN_TILE0"][wi], meta["T_W"][wi]
            sl = slot_eid[c][t0:t0 + tw]
            va = sl >= 0
            if va.any():
                DMAX = max(DMAX, len(np.unique(edge_src[sl[va]])))
    return DMAX


def _stage_core(c, meta, slots, inputs, DMAX):
    v = np.asarray(inputs["v"], np.float32)
    e = np.asarray(inputs["e"], np.float32)
    edge_src = np.asarray(inputs["edge_src"]).astype(np.int64)
    edge_dst = np.asarray(inputs["edge_dst"]).astype(np.int64)
    T_TOT, BASE, TWIN = meta["T_TOT"], meta["BASE"], meta["TWIN"]

    valid = slots >= 0                     # [T_TOT, 128]
    eids = np.where(valid, slots, 0)

    e_fm = np.zeros((65, T_TOT * 128), np.float32)
    cols = np.arange(T_TOT)[:, None] * 128 + np.arange(128)[None, :]
    e_fm[:64, cols[valid]] = e[eids[valid]].T
    e_fm[64, :] = 1.0

    S = np.zeros((128, T_TOT * SW), np.float32)
    wstart = np.array(WIN_STARTS)[TWIN]
    d_local = edge_dst[eids] - c * NC - wstart[:, None]
    off = d_local - BASE[:, None]
    tv, sv = np.nonzero(valid)
    offv = off[tv, sv]
    assert (offv >= 0).all() and (offv < SW).all()
    S[sv, tv * SW + offv] = 1.0

    # v_src in tile-slot order: [128 slots, T_TOT*128] (partition = slot)
    srcs = edge_src[eids]                       # [T_TOT, 128]
    vsrc = v[srcs].astype(BF16)                 # [T_TOT, 128, VD]
    vsrc = np.ascontiguousarray(vsrc.transpose(1, 0, 2)).reshape(128, T_TOT * VD)

    return dict(e_fm=e_fm.astype(BF16), S=S.astype(BF16), vsrc=vsrc)


def _stage_weights(inputs):
    f32 = lambda k: np.asarray(inputs[k], np.float32)
    cols, colmap = [], {}

    def addw(name, arr):
        colmap[name] = sum(a.shape[1] for a in cols)
        cols.append(arr)

    addw("A", f32("A_w"))
    addw("C", f32("C_w"))
    addw("mA", np.concatenate([f32("mA_w")[k] for k in range(KH)], axis=1))
    addw("mB", np.concatenate([f32("mB_w")[k] for k in range(KH)], axis=1))
    addw("mD", np.concatenate([f32("mD_w")[k] for k in range(KH)], axis=1))
    addw("B", np.concatenate([f32("B_w")[128 * k:128 * (k + 1)] for k in range(KH)], axis=1))
    addw("E", np.concatenate([f32("E_w")[:128], f32("E_w")[128:]], axis=1))
    addw("gmA", f32("gm_A_w"))
    addw("gmB", f32("gm_B_w"))
    addw("gmWih", f32("gm_Wih"))
    addw("gmWhh", f32("gm_Whh"))
    addw("gsA", f32("gs_A_w"))
    addw("gsB", f32("gs_B_w"))
    addw("gsWih", f32("gs_Wih"))
    addw("gsWhh", f32("gs_Whh"))
    wpack = np.concatenate(cols, axis=1).astype(BF16)

    bcols, bmap = [], {}

    def addb(name, vec):
        bmap[name] = len(bcols)
        bcols.append(np.asarray(vec, np.float32).reshape(128))

    addb("A_b", f32("A_b"))
    addb("C_b", f32("C_b"))
    addb("E_b", f32("E_b"))
    addb("B_b", f32("B_b") + f32("mD_b").reshape(KH * H) @ f32("B_w"))
    addb("gmz", f32("gm_A_b") + f32("gm_B_b"))
    addb("gm_br", f32("gm_bih")[:128] + f32("gm_bhh")[:128])
    addb("gm_bz", f32("gm_bih")[128:256] + f32("gm_bhh")[128:256])
    addb("gm_bihn", f32("gm_bih")[256:])
    addb("gm_bhhn", f32("gm_bhh")[256:])
    addb("gsz", f32("gs_A_b") + f32("gs_B_b"))
    addb("gs_br", f32("gs_bih")[:128] + f32("gs_bhh")[:128])
    addb("gs_bz", f32("gs_bih")[128:256] + f32("gs_bhh")[128:256])
    addb("gs_bihn", f32("gs_bih")[256:])
    addb("gs_bhhn", f32("gs_bhh")[256:])
    for k in range(KH):
        addb(f"mA_b{k}", f32("mA_b")[k])
    for k in range(KH):
        addb(f"mB_b{k}", f32("mB_b")[k])
    for k in range(KH):
        addb(f"mC_{k}", f32("mC_w")[k])
    biases = np.stack(bcols, axis=1)

    kwb = np.concatenate([f32("K_w"), f32("K_b")[None, :]], axis=0).astype(BF16)
    return wpack, colmap, biases, bmap, kwb


# ===========================================================================
# Program builder
# ===========================================================================

def build_program(meta, DMAX):
    T_TOT, T_W, WIN_TILE0 = meta["T_TOT"], meta["T_W"], meta["WIN_TILE0"]
    BASE, TWIN = meta["BASE"], meta["TWIN"]
    NSEG, SEG_W0, SEG_WSPAN = meta["NSEG"], meta["SEG_W0"], meta["SEG_WSPAN"]
    COLMAP, BMAP, NB = meta["COLMAP"], meta["BMAP"], meta["NB"]
    WPACK_COLS = meta["WPACK_COLS"]

    nc = bacc.Bacc("TRN2")
    d_vsrc = nc.dram_tensor("vsrc", [128, T_TOT * VD], dt.bfloat16, kind="ExternalInput")
    d_efm = nc.dram_tensor("e_fm", [65, T_TOT * 128], dt.bfloat16, kind="ExternalInput")
    d_S = nc.dram_tensor("S", [128, T_TOT * SW], dt.bfloat16, kind="ExternalInput")
    d_vfm = nc.dram_tensor("v_fm", [128, NC], dt.bfloat16, kind="ExternalInput")
    d_sfm = nc.dram_tensor("s_fm", [128, GC], dt.bfloat16, kind="ExternalInput")
    d_w = nc.dram_tensor("wpack", [128, WPACK_COLS], dt.bfloat16, kind="ExternalInput")
    d_kwb = nc.dram_tensor("kwb", [65, 128], dt.bfloat16, kind="ExternalInput")
    d_bias = nc.dram_tensor("biases", [128, NB], dt.float32, kind="ExternalInput")
    d_oh = nc.dram_tensor("onehot", [4, KH * 128], dt.bfloat16, kind="ExternalInput")
    d_outv = nc.dram_tensor("out_v", [128, NC], dt.float32, kind="ExternalOutput")
    d_outs = nc.dram_tensor("out_s", [128, GC], dt.float32, kind="ExternalOutput")

    with tile.TileContext(nc) as tc:
        with (
            tc.tile_pool(name="const", bufs=1) as cpool,
            tc.tile_pool(name="pers", bufs=1) as ppool,
            tc.tile_pool(name="ewin", bufs=2) as ewin,
            tc.tile_pool(name="chk", bufs=2) as chk,
            tc.tile_pool(name="dn", bufs=2) as dnp,
            tc.tile_pool(name="psw", bufs=2, space="PSUM") as psw,
            tc.tile_pool(name="pske", bufs=2, space="PSUM") as pske,
            tc.tile_pool(name="pssm", bufs=3, space="PSUM") as pssm,
            tc.tile_pool(name="psa", bufs=1, space="PSUM") as psa,
        ):
            # ---------------- constants
            w_sb = cpool.tile([128, WPACK_COLS], dt.bfloat16)
            nc.sync.dma_start(w_sb[:], d_w[:])
            kwb_sb = cpool.tile([65, 128], dt.bfloat16)
            nc.sync.dma_start(kwb_sb[:], d_kwb[:])
            bias_sb = cpool.tile([128, NB], dt.float32)
            nc.sync.dma_start(bias_sb[:], d_bias[:])
            vfm_sb = ppool.tile([128, NC], dt.bfloat16)
            nc.sync.dma_start(vfm_sb[:], d_vfm[:])
            sfm_sb = cpool.tile([128, GC], dt.bfloat16)
            nc.sync.dma_start(sfm_sb[:], d_sfm[:])

            def W(name, j0=0, w=128):
                o = COLMAP[name]
                return w_sb[:, o + j0: o + j0 + w]

            def B(name):
                return bias_sb[:, BMAP[name]:BMAP[name] + 1]

            # ---------------- persistent intermediates
            sve_sb = ppool.tile([128, NC], dt.bfloat16)
            aexp_sb = ppool.tile([4, GC, RG], dt.bfloat16)
            pooled_sb = ppool.tile([128, KH, GC], dt.float32)
            pooled_bf = ppool.tile([128, KH * GC], dt.bfloat16)
            s2m_sb = cpool.tile([128, GC], dt.bfloat16)
            denom_sb = cpool.tile([4, GC], dt.float32)
            rden_sb = cpool.tile([4, GC], dt.bfloat16)

            # s2m = tanh(C^T s + C_b)
            ps = pssm.tile([128, CHUNK], dt.float32, tag="ps500")
            nc.tensor.matmul(ps[:, :GC], W("C"), sfm_sb[:], start=True, stop=True)
            nc.scalar.activation(s2m_sb[:], ps[:, :GC], Act.Tanh, bias=B("C_b"))

            # ---------------- P1: edge pipeline (gather segments x windows)
            pw = None
            cur_w = -1
            for si in range(NSEG):
                t0, t1 = si * SEGT, min((si + 1) * SEGT, T_TOT)
                nt = t1 - t0
                e_sb = ewin.tile([65, nt * 128], dt.bfloat16, tag="e_sb")
                nc.sync.dma_start(e_sb[:], d_efm[:, t0 * 128: t1 * 128])
                s_sb = ewin.tile([128, nt * SW], dt.bfloat16, tag="s_sb")
                nc.sync.dma_start(s_sb[:], d_S[:, t0 * SW: t1 * SW])
                vsrc = ewin.tile([128, nt, 128], dt.bfloat16, tag="vsrc")
                nc.sync.dma_start(
                    vsrc[:].rearrange("p a b -> p (a b)"),
                    d_vsrc[:, t0 * VD: t1 * VD])
                ve = ewin.tile([128, nt, 128], dt.bfloat16, tag="ve")
                for s0 in range(0, nt, SUB):
                    s1 = min(s0 + SUB, nt)
                    kep = pske.tile([128, SUB * 128], dt.float32, tag="kep")
                    for t in range(s0, s1):
                        nc.tensor.matmul(
                            kep[:, (t - s0) * 128: (t - s0 + 1) * 128],
                            e_sb[:, t * 128: (t + 1) * 128],
                            kwb_sb[:], start=True, stop=True)
                    ncols = (s1 - s0) * 128
                    vep = ewin.tile([128, SUB * 128], dt.bfloat16, tag="vep")
                    nc.vector.tensor_tensor(
                        vep[:, :ncols], kep[:, :ncols],
                        vsrc[:, s0:s1, :].rearrange("p a b -> p (a b)"),
                        op=Alu.mult)
                    nc.scalar.activation(
                        ve[:, s0:s1, :].rearrange("p a b -> p (a b)"),
                        vep[:, :ncols], Act.Prelu, alpha=0.1)
                # scatter-accumulate, handling window boundaries
                for t in range(t0, t1):
                    wi = int(TWIN[t])
                    if wi != cur_w:
                        if cur_w >= 0:
                            nc.vector.tensor_copy(
                                sve_sb[:, WIN_STARTS[cur_w]:
                                       WIN_STARTS[cur_w] + WIN_SIZES[cur_w]],
                                pw[:, :WIN_SIZES[cur_w]])
                        pw = psw.tile([128, WIN], dt.float32, tag="pw")
                        nc.vector.memset(pw[:], 0.0)
                        cur_w = wi
                    b = int(BASE[t])
                    nc.tensor.matmul(
                        pw[:, b: b + SW], ve[:, t - t0, :],
                        s_sb[:, (t - t0) * SW: (t - t0 + 1) * SW],
                        start=False, stop=(t == T_TOT - 1 or TWIN[t + 1] != wi),
                        skip_group_check=True)
            nc.vector.tensor_copy(
                sve_sb[:, WIN_STARTS[cur_w]: WIN_STARTS[cur_w] + WIN_SIZES[cur_w]],
                pw[:, :WIN_SIZES[cur_w]])

            # ---------------- P2 prologue: d_super, Wmat (padded per head)
            wmatp = []
            for k in range(KH):
                dsp = pssm.tile([128, CHUNK], dt.float32, tag="ps500")
                nc.tensor.matmul(dsp[:, :GC], W("mB", k * 128), sfm_sb[:],
                                 start=True, stop=True)
                ds_k = cpool.tile([128, GC], dt.bfloat16, tag=f"ds{k}")
                nc.scalar.activation(ds_k[:], dsp[:, :GC], Act.Tanh, bias=B(f"mB_b{k}"))
                wp = cpool.tile([128, GC, KH], dt.bfloat16, tag=f"wmatp{k}")
                nc.vector.memset(wp[:], 0.0)
                nc.vector.tensor_scalar(wp[:, :, k], ds_k[:], B(f"mC_{k}"), None,
                                        op0=Alu.mult)
                wmatp.append(wp)

            # ---------------- P2: attention
            for ci in range(NCH):
                nsl = slice(ci * CHUNK, (ci + 1) * CHUNK)
                dns = []
                for k in range(KH):
                    dnps = pssm.tile([128, CHUNK], dt.float32, tag="ps500")
                    nc.tensor.matmul(dnps[:], W("mA", k * 128), vfm_sb[:, nsl],
                                     start=True, stop=True)
                    dn_k = dnp.tile([128, CHUNK], dt.bfloat16, tag=f"dn{k}")
                    nc.scalar.activation(dn_k[:], dnps[:], Act.Tanh,
                                         bias=B(f"mA_b{k}"))
                    dns.append(dn_k)
                aps = psa.tile([4, CHUNK], dt.float32, tag="aps")
                for gl in range(GPC):
                    g = ci * GPC + gl
                    for k in range(KH):
                        nc.tensor.matmul(
                            aps[:, gl * RG: (gl + 1) * RG],
                            wmatp[k][:, g, :],
                            dns[k][:, gl * RG: (gl + 1) * RG],
                            start=(k == 0), stop=(k == KH - 1))
                nc.scalar.activation(
                    aexp_sb[:, ci * GPC:(ci + 1) * GPC, :],
                    aps[:].rearrange("p (a b) -> p a b", a=GPC), Act.Exp)

            nc.vector.tensor_reduce(denom_sb[:], aexp_sb[:], axis=Axis.X, op=Alu.add)
            with nc.allow_low_precision(reason="bf16 recip ok at 2e-2 gate"):
                nc.vector.reciprocal(rden_sb[:], denom_sb[:])

            # one-hot rows for PE partition-broadcast (row k of a 4-row
            # tensor -> all 128 partitions): oh[c, 128k+h] = (c == k)
            oh_sb = cpool.tile([4, KH * 128], dt.bfloat16)
            nc.sync.dma_start(oh_sb[:], d_oh[:])

            for ci in range(NCH):
                nsl = slice(ci * CHUNK, (ci + 1) * CHUNK)
                for k in range(KH):
                    abp = pssm.tile([128, CHUNK], dt.float32, tag="ps500")
                    nc.tensor.matmul(
                        abp[:].rearrange("p (a b) -> p a b", a=GPC),
                        oh_sb[:, k * 128:(k + 1) * 128],
                        aexp_sb[0:4, ci * GPC:(ci + 1) * GPC, :],
                        start=True, stop=True)
                    ab = chk.tile([128, CHUNK], dt.bfloat16, tag="ab")
                    nc.scalar.activation(ab[:], abp[:], Act.Copy)
                    dvp = pssm.tile([128, CHUNK], dt.float32, tag="ps500")
                    nc.tensor.matmul(dvp[:], W("mD", k * 128), vfm_sb[:, nsl],
                                     start=True, stop=True)
                    dvw = chk.tile([128, GPC, RG], dt.bfloat16, tag="dvw")
                    nc.vector.tensor_tensor(
                        dvw[:].rearrange("p a b -> p (a b)"), dvp[:], ab[:],
                        op=Alu.mult)
                    nc.vector.tensor_reduce(
                        pooled_sb[:, k, ci * GPC:(ci + 1) * GPC], dvw[:],
                        axis=Axis.X, op=Alu.add)
            # pooled_bf = pooled_raw * rden (broadcast rden rows via PE)
            rbp = pssm.tile([128, CHUNK], dt.float32, tag="ps500")
            for k in range(KH):
                nc.tensor.matmul(rbp[:, k * GC:(k + 1) * GC],
                                 oh_sb[:, k * 128:(k + 1) * 128],
                                 rden_sb[0:4, :], start=True, stop=True)
            rden_bc = chk.tile([128, KH * GC], dt.bfloat16, tag="rdenbc")
            nc.scalar.activation(rden_bc[:], rbp[:], Act.Copy)
            nc.vector.tensor_tensor(pooled_bf[:],
                                    pooled_sb[:].rearrange("p a b -> p (a b)"),
                                    rden_bc[:], op=Alu.mult)

            # ---------------- P3: message + GRU per node chunk
            for ci in range(NCH):
                nsl = slice(ci * CHUNK, (ci + 1) * CHUNK)
                gsl = slice(ci * GPC, (ci + 1) * GPC)
                s2m_b3 = s2m_sb[:, gsl].unsqueeze(2).to_broadcast([128, GPC, RG])

                def r3(ap):
                    return ap.rearrange("p (a b) -> p a b", a=GPC)

                mp = pssm.tile([128, CHUNK], dt.float32, tag="ps500")
                nc.tensor.matmul(mp[:], W("E", 0), sve_sb[:, nsl], start=True, stop=False)
                nc.tensor.matmul(mp[:], W("E", 128), vfm_sb[:, nsl], start=False, stop=True)
                m2m = chk.tile([128, CHUNK], dt.bfloat16, tag="m2m")
                nc.scalar.activation(m2m[:], mp[:], Act.Prelu, bias=B("E_b"), alpha=0.1)

                zp = pssm.tile([128, CHUNK], dt.float32, tag="ps500")
                nc.tensor.matmul(zp[:], W("gmA"), m2m[:], start=True, stop=False)
                nc.tensor.matmul(zp[:].rearrange("p (a b) -> p a b", a=GPC),
                                 W("gmB"), s2m_b3, start=False, stop=True)
                zt = chk.tile([128, CHUNK], dt.bfloat16, tag="zt")
                nc.scalar.activation(zt[:], zp[:], Act.Sigmoid, bias=B("gmz"))

                t1 = chk.tile([128, CHUNK], dt.bfloat16, tag="t1")
                nc.vector.tensor_tensor(r3(t1[:]), s2m_b3, r3(m2m[:]), op=Alu.subtract)
                t2 = chk.tile([128, CHUNK], dt.bfloat16, tag="t2")
                nc.vector.tensor_tensor(t2[:], zt[:], t1[:], op=Alu.mult)
                hm = chk.tile([128, CHUNK], dt.bfloat16, tag="hm")
                nc.vector.tensor_tensor(hm[:], t2[:], m2m[:], op=Alu.add)

                rp = pssm.tile([128, CHUNK], dt.float32, tag="ps500")
                nc.tensor.matmul(rp[:], W("gmWih", 0), vfm_sb[:, nsl], start=True, stop=False)
                nc.tensor.matmul(rp[:], W("gmWhh", 0), hm[:], start=False, stop=True)
                r = chk.tile([128, CHUNK], dt.bfloat16, tag="r")
                nc.scalar.activation(r[:], rp[:], Act.Sigmoid, bias=B("gm_br"))

                z2p = pssm.tile([128, CHUNK], dt.float32, tag="ps500")
                nc.tensor.matmul(z2p[:], W("gmWih", 128), vfm_sb[:, nsl], start=True, stop=False)
                nc.tensor.matmul(z2p[:], W("gmWhh", 128), hm[:], start=False, stop=True)
                z2 = chk.tile([128, CHUNK], dt.bfloat16, tag="z2")
                nc.scalar.activation(z2[:], z2p[:], Act.Sigmoid, bias=B("gm_bz"))

                innp = pssm.tile([128, CHUNK], dt.float32, tag="ps500")
                nc.tensor.matmul(innp[:], W("gmWih", 256), vfm_sb[:, nsl], start=True, stop=True)
                hnp = pssm.tile([128, CHUNK], dt.float32, tag="ps500")
                nc.tensor.matmul(hnp[:], W("gmWhh", 256), hm[:], start=True, stop=True)
                t3 = chk.tile([128, CHUNK], dt.float32, tag="t3")
                nc.vector.scalar_tensor_tensor(t3[:], hnp[:], B("gm_bhhn"), r[:],
                                               op0=Alu.add, op1=Alu.mult)
                t4 = chk.tile([128, CHUNK], dt.float32, tag="t4")
                nc.vector.tensor_tensor(t4[:], t3[:], innp[:], op=Alu.add)
                n = chk.tile([128, CHUNK], dt.bfloat16, tag="n")
                nc.scalar.activation(n[:], t4[:], Act.Tanh, bias=B("gm_bihn"))
                u1 = chk.tile([128, CHUNK], dt.bfloat16, tag="u1")
                nc.vector.tensor_tensor(u1[:], hm[:], n[:], op=Alu.subtract)
                u2 = chk.tile([128, CHUNK], dt.bfloat16, tag="u2")
                nc.vector.tensor_tensor(u2[:], z2[:], u1[:], op=Alu.mult)
                uo = chk.tile([128, CHUNK], dt.float32, tag="uo")
                nc.vector.tensor_tensor(uo[:], u2[:], n[:], op=Alu.add)
                nc.sync.dma_start(d_outv[:, nsl], uo[:])

            # ---------------- P4: supernode side (125 graphs at once)
            def gmm(w1, r1, w2=None, r2=None):
                p = pssm.tile([128, CHUNK], dt.float32, tag="ps500")
                nc.tensor.matmul(p[:, :GC], w1, r1, start=True, stop=(w2 is None))
                if w2 is not None:
                    nc.tensor.matmul(p[:, :GC], w2, r2, start=False, stop=True)
                return p

            def act(p, func, bias, tag):
                o = chk.tile([128, GC], dt.bfloat16, tag=tag)
                nc.scalar.activation(o[:], p[:, :GC], func, bias=B(bias))
                return o

            s2s = act(gmm(W("A"), sfm_sb[:]), Act.Tanh, "A_b", "s2s")
            p = pssm.tile([128, CHUNK], dt.float32, tag="ps500")
            for k in range(KH):
                nc.tensor.matmul(p[:, :GC], W("B", k * 128),
                                 pooled_bf[:, k * GC:(k + 1) * GC],
                                 start=(k == 0), stop=(k == KH - 1))
            m2s = chk.tile([128, GC], dt.bfloat16, tag="m2s")
            nc.scalar.activation(m2s[:], p[:, :GC], Act.Tanh, bias=B("B_b"))
            zs = act(gmm(W("gsA"), s2s[:], W("gsB"), m2s[:]), Act.Sigmoid, "gsz", "zs")
            st1 = chk.tile([128, GC], dt.bfloat16, tag="st1")
            nc.vector.tensor_tensor(st1[:], m2s[:], s2s[:], op=Alu.subtract)
            st2 = chk.tile([128, GC], dt.bfloat16, tag="st2")
            nc.vector.tensor_tensor(st2[:], zs[:], st1[:], op=Alu.mult)
            hs = chk.tile([128, GC], dt.bfloat16, tag="hs")
            nc.vector.tensor_tensor(hs[:], st2[:], s2s[:], op=Alu.add)
            rs = act(gmm(W("gsWih", 0), sfm_sb[:], W("gsWhh", 0), hs[:]),
                     Act.Sigmoid, "gs_br", "rs")
            z2s = act(gmm(W("gsWih", 128), sfm_sb[:], W("gsWhh", 128), hs[:]),
                      Act.Sigmoid, "gs_bz", "z2s")
            innp = gmm(W("gsWih", 256), sfm_sb[:])
            hnp = gmm(W("gsWhh", 256), hs[:])
            st3 = chk.tile([128, GC], dt.float32, tag="st3")
            nc.vector.scalar_tensor_tensor(st3[:], hnp[:, :GC], B("gs_bhhn"), rs[:],
                                           op0=Alu.add, op1=Alu.mult)
            st4 = chk.tile([128, GC], dt.float32, tag="st4")
            nc.vector.tensor_tensor(st4[:], st3[:], innp[:, :GC], op=Alu.add)
            ns = chk.tile([128, GC], dt.bfloat16, tag="ns")
            nc.scalar.activation(ns[:], st4[:], Act.Tanh, bias=B("gs_bihn"))
            su1 = chk.tile([128, GC], dt.bfloat16, tag="su1")
            nc.vector.tensor_tensor(su1[:], hs[:], ns[:], op=Alu.subtract)
            su2 = chk.tile([128, GC], dt.bfloat16, tag="su2")
            nc.vector.tensor_tensor(su2[:], z2s[:], su1[:], op=Alu.mult)
            suo = chk.tile([128, GC], dt.float32, tag="suo")
            nc.vector.tensor_tensor(suo[:], su2[:], ns[:], op=Alu.add)
            nc.sync.dma_start(d_outs[:], suo[:])

    nc.compile()
    return nc


# ===========================================================================
# Entry point
# ===========================================================================

def prepare(inputs):
    meta, slot_eid = _pack_edges(inputs["edge_src"], inputs["edge_dst"])
    DMAX = _compute_dmax(meta, slot_eid,
                         np.asarray(inputs["edge_src"]).astype(np.int64))
    wpack, colmap, biases, bmap, kwb = _stage_weights(inputs)
    meta["WPACK_COLS"] = wpack.shape[1]
    meta["COLMAP"] = colmap
    meta["BMAP"] = bmap
    meta["NB"] = biases.shape[1]

    v = np.asarray(inputs["v"], np.float32)
    s = np.asarray(inputs["s"], np.float32)
    onehot = np.zeros((4, KH * 128), np.float32)
    for k in range(KH):
        onehot[k, k * 128:(k + 1) * 128] = 1.0
    onehot = onehot.astype(BF16)
    in_maps = []
    for c in range(NCORES):
        st = _stage_core(c, meta, slot_eid[c], inputs, DMAX)
        in_maps.append(dict(
            vsrc=np.ascontiguousarray(st["vsrc"]),
            e_fm=np.ascontiguousarray(st["e_fm"]),
            S=np.ascontiguousarray(st["S"]),
            v_fm=np.ascontiguousarray(v[c * NC:(c + 1) * NC].T).astype(BF16),
            s_fm=np.ascontiguousarray(s[c * GC:(c + 1) * GC].T).astype(BF16),
            wpack=wpack, kwb=kwb, biases=biases, onehot=onehot,
        ))
    return meta, DMAX, in_maps


def kernel(**inputs):
    meta, DMAX, in_maps = prepare(inputs)
    nc = build_program(meta, DMAX)

    from concourse.bass_utils import run_bass_kernel_spmd
    res = run_bass_kernel_spmd(nc, in_maps, core_ids=list(range(NCORES)))
    upd_v = np.concatenate(
        [np.asarray(res.results[c]["out_v"]).T for c in range(NCORES)], axis=0)
    upd_s = np.concatenate(
        [np.asarray(res.results[c]["out_s"]).T for c in range(NCORES)], axis=0)
    return upd_v.astype(np.float32), upd_s.astype(np.float32)


# revision 15
# speedup vs baseline: 2.2383x; 1.0331x over previous
"""Adaptive-GCN Trainium2 kernel: 8-core data-parallel Bass implementation.

Sharding: graphs/nodes are split contiguously across 8 cores (125 graphs /
12500 nodes per core).  Edges are assigned to the core owning their
destination node; the v[edge_src] gather reads per-window deduplicated
v-row tables staged in each core's DRAM, so execution needs no cross-core
communication.

Edges are re-packed on the host into 128-edge tiles grouped by 512-node
destination windows.  Tile structure (counts, node-base offsets) is
synchronized across cores so one SPMD program fits all cores; per-core
variability lives in DMA'd data (edge features, gather indices, 0/1
scatter matrices S).

  ke  = e_tile^T @ [K_w; K_b]      (PE; edge features are the stationary side)
  ve  = leakyrelu(ke * v[src])     (DVE multiply, ACT Lrelu)
  sve = sum_tiles  ve_tile^T @ S   (PE, accumulated into a PSUM node-window)

Node/graph phases run feature-major ([128 features x nodes]) with bf16
matmul operands and f32 PSUM accumulation.
"""
import sys

sys.path.insert(0, "/opt/trn_rl_repo")

import numpy as np
import ml_dtypes

from concourse import bacc, mybir, tile
import concourse.bass_utils as _bu
import concourse.tile_utils as _tu

_orig_gwa = _bu.get_walrus_args


def _gwa(*a, **k):
    cmd = _orig_gwa(*a, **k)
    return ["--enable-ldw-opt=true" if c == "--enable-ldw-opt=false" else c
            for c in cmd]


_bu.get_walrus_args = _gwa

_tu.max_sbuf_usage = 200 * 1024  # stale 192K cap; cayman has 208K usable

dt = mybir.dt
Alu = mybir.AluOpType
Act = mybir.ActivationFunctionType
Axis = mybir.AxisListType
BF16 = ml_dtypes.bfloat16

# ---------------- problem constants (must match reference.py) --------------
NCORES = 8
N, NE, G = 100_000, 800_000, 1_000
VD, ED, H, KH = 128, 64, 128, 4
NC = N // NCORES          # 12500 nodes per core
GC = G // NCORES          # 125 graphs per core
RG = N // G               # 100 nodes per graph
WIN = 512                 # scatter window (one PSUM bank of f32)
SW = 64                   # S-matrix node span per edge tile
SUB = 4                   # edge tiles per elementwise batch (512 edges)
SEGT = 32                 # edge tiles per gather call / edge-stream buffer
CHUNK = 500               # node chunk for phases 2/3 (5 graphs)
NCH = NC // CHUNK         # 25 chunks
GPC = CHUNK // RG         # graphs per chunk (5)

WIN_STARTS = list(range(0, NC, WIN))
WIN_SIZES = [min(WIN, NC - s) for s in WIN_STARTS]
NWIN = len(WIN_STARTS)


# ===========================================================================
# Host-side sharding / packing
# ===========================================================================

def _pack_edges(edge_src, edge_dst):
    """Assign edges to cores by dst; pack into synchronized 128-edge tiles."""
    edge_dst = np.asarray(edge_dst).astype(np.int64)
    core_of = edge_dst // NC

    per_cw = [[None] * NWIN for _ in range(NCORES)]
    for c in range(NCORES):
        m = np.nonzero(core_of == c)[0]
        d = edge_dst[m] - c * NC
        order = np.argsort(d, kind="stable")
        m, d = m[order], d[order]
        for wi in range(NWIN):
            lo = np.searchsorted(d, WIN_STARTS[wi], side="left")
            hi = np.searchsorted(d, WIN_STARTS[wi] + WIN_SIZES[wi], side="left")
            per_cw[c][wi] = (m[lo:hi], d[lo:hi] - WIN_STARTS[wi])

    tiles_base, tiles_window, win_tile0 = [], [], []
    slot_eid = [[] for _ in range(NCORES)]
    for wi in range(NWIN):
        wsz = WIN_SIZES[wi]
        ptr = [0] * NCORES
        arrs = [per_cw[c][wi] for c in range(NCORES)]
        win_tile0.append(len(tiles_base))
        while True:
            active = [c for c in range(NCORES) if ptr[c] < len(arrs[c][0])]
            if not active:
                break
            base = min(int(arrs[c][1][ptr[c]]) for c in active)
            base = min(base, max(0, wsz - SW))
            for c in range(NCORES):
                eids, d = arrs[c]
                j0 = ptr[c]
                hi = np.searchsorted(d, base + SW, side="left")
                take = min(hi - j0, 128)
                sl = np.full(128, -1, dtype=np.int64)
                sl[:take] = eids[j0:j0 + take]
                slot_eid[c].append(sl)
                ptr[c] = j0 + take
            tiles_base.append(base)
            tiles_window.append(wi)
        if win_tile0[-1] == len(tiles_base):
            for c in range(NCORES):
                slot_eid[c].append(np.full(128, -1, dtype=np.int64))
            tiles_base.append(0)
            tiles_window.append(wi)

    T_TOT = len(tiles_base)
    T_W = [0] * NWIN
    for wi in tiles_window:
        T_W[wi] += 1
    meta = dict(T_TOT=T_TOT, T_W=T_W, WIN_TILE0=win_tile0,
                BASE=np.array(tiles_base), TWIN=np.array(tiles_window))
    # gather segments: SEGT tiles each; record base window + window span
    nseg = (T_TOT + SEGT - 1) // SEGT
    seg_w0, seg_wspan = [], []
    for si in range(nseg):
        t0, t1 = si * SEGT, min((si + 1) * SEGT, T_TOT)
        ws = [tiles_window[t] for t in range(t0, t1)]
        seg_w0.append(min(ws))
        seg_wspan.append(max(ws) - min(ws) + 1)
    meta["NSEG"] = nseg
    meta["SEG_W0"] = seg_w0
    meta["SEG_WSPAN"] = seg_wspan
    return meta, [np.stack(s) for s in slot_eid]


def _compute_dmax(meta, slot_eid, edge_src):
    DMAX = 1
    for c in range(NCORES):
        for wi in range(NWIN):
            t0, tw = meta["WIN_TILE0"][wi], meta["T_W"][wi]
            sl = slot_eid[c][t0:t0 + tw]
            va = sl >= 0
            if va.any():
                DMAX = max(DMAX, len(np.unique(edge_src[sl[va]])))
    return DMAX


def _stage_core(c, meta, slots, inputs, DMAX):
    v = np.asarray(inputs["v"], np.float32)
    e = np.asarray(inputs["e"], np.float32)
    edge_src = np.asarray(inputs["edge_src"]).astype(np.int64)
    edge_dst = np.asarray(inputs["edge_dst"]).astype(np.int64)
    T_TOT, BASE, TWIN = meta["T_TOT"], meta["BASE"], meta["TWIN"]

    valid = slots >= 0                     # [T_TOT, 128]
    eids = np.where(valid, slots, 0)

    e_fm = np.zeros((65, T_TOT * 128), np.float32)
    cols = np.arange(T_TOT)[:, None] * 128 + np.arange(128)[None, :]
    e_fm[:64, cols[valid]] = e[eids[valid]].T
    e_fm[64, :] = 1.0

    S = np.zeros((128, T_TOT * SW), np.float32)
    wstart = np.array(WIN_STARTS)[TWIN]
    d_local = edge_dst[eids] - c * NC - wstart[:, None]
    off = d_local - BASE[:, None]
    tv, sv = np.nonzero(valid)
    offv = off[tv, sv]
    assert (offv >= 0).all() and (offv < SW).all()
    S[sv, tv * SW + offv] = 1.0

    # v_src in tile-slot order: [128 slots, T_TOT*128] (partition = slot)
    srcs = edge_src[eids]                       # [T_TOT, 128]
    vsrc = v[srcs].astype(BF16)                 # [T_TOT, 128, VD]
    vsrc = np.ascontiguousarray(vsrc.transpose(1, 0, 2)).reshape(128, T_TOT * VD)

    return dict(e_fm=e_fm.astype(BF16), S=S.astype(BF16), vsrc=vsrc)


def _stage_weights(inputs):
    f32 = lambda k: np.asarray(inputs[k], np.float32)
    cols, colmap = [], {}

    def addw(name, arr):
        colmap[name] = sum(a.shape[1] for a in cols)
        cols.append(arr)

    addw("A", f32("A_w"))
    addw("C", f32("C_w"))
    addw("mA", np.concatenate([f32("mA_w")[k] for k in range(KH)], axis=1))
    addw("mB", np.concatenate([f32("mB_w")[k] for k in range(KH)], axis=1))
    addw("mD", np.concatenate([f32("mD_w")[k] for k in range(KH)], axis=1))
    addw("B", np.concatenate([f32("B_w")[128 * k:128 * (k + 1)] for k in range(KH)], axis=1))
    addw("E", np.concatenate([f32("E_w")[:128], f32("E_w")[128:]], axis=1))
    addw("gmA", f32("gm_A_w"))
    addw("gmB", f32("gm_B_w"))
    addw("gmWih", f32("gm_Wih"))
    addw("gmWhh", f32("gm_Whh"))
    addw("gsA", f32("gs_A_w"))
    addw("gsB", f32("gs_B_w"))
    addw("gsWih", f32("gs_Wih"))
    addw("gsWhh", f32("gs_Whh"))
    wpack = np.concatenate(cols, axis=1).astype(BF16)

    bcols, bmap = [], {}

    def addb(name, vec):
        bmap[name] = len(bcols)
        bcols.append(np.asarray(vec, np.float32).reshape(128))

    addb("A_b", f32("A_b"))
    addb("C_b", f32("C_b"))
    addb("E_b", f32("E_b"))
    addb("B_b", f32("B_b") + f32("mD_b").reshape(KH * H) @ f32("B_w"))
    addb("gmz", f32("gm_A_b") + f32("gm_B_b"))
    addb("gm_br", f32("gm_bih")[:128] + f32("gm_bhh")[:128])
    addb("gm_bz", f32("gm_bih")[128:256] + f32("gm_bhh")[128:256])
    addb("gm_bihn", f32("gm_bih")[256:])
    addb("gm_bhhn", f32("gm_bhh")[256:])
    addb("gsz", f32("gs_A_b") + f32("gs_B_b"))
    addb("gs_br", f32("gs_bih")[:128] + f32("gs_bhh")[:128])
    addb("gs_bz", f32("gs_bih")[128:256] + f32("gs_bhh")[128:256])
    addb("gs_bihn", f32("gs_bih")[256:])
    addb("gs_bhhn", f32("gs_bhh")[256:])
    for k in range(KH):
        addb(f"mA_b{k}", f32("mA_b")[k])
    for k in range(KH):
        addb(f"mB_b{k}", f32("mB_b")[k])
    for k in range(KH):
        addb(f"mC_{k}", f32("mC_w")[k])
    biases = np.stack(bcols, axis=1)

    kwb = np.concatenate([f32("K_w"), f32("K_b")[None, :]], axis=0).astype(BF16)
    return wpack, colmap, biases, bmap, kwb


# ===========================================================================
# Program builder
# ===========================================================================

def build_program(meta, DMAX):
    T_TOT, T_W, WIN_TILE0 = meta["T_TOT"], meta["T_W"], meta["WIN_TILE0"]
    BASE, TWIN = meta["BASE"], meta["TWIN"]
    NSEG, SEG_W0, SEG_WSPAN = meta["NSEG"], meta["SEG_W0"], meta["SEG_WSPAN"]
    COLMAP, BMAP, NB = meta["COLMAP"], meta["BMAP"], meta["NB"]
    WPACK_COLS = meta["WPACK_COLS"]

    nc = bacc.Bacc("TRN2")
    d_vsrc = nc.dram_tensor("vsrc", [128, T_TOT * VD], dt.bfloat16, kind="ExternalInput")
    d_efm = nc.dram_tensor("e_fm", [65, T_TOT * 128], dt.bfloat16, kind="ExternalInput")
    d_S = nc.dram_tensor("S", [128, T_TOT * SW], dt.bfloat16, kind="ExternalInput")
    d_vfm = nc.dram_tensor("v_fm", [128, NC], dt.bfloat16, kind="ExternalInput")
    d_sfm = nc.dram_tensor("s_fm", [128, GC], dt.bfloat16, kind="ExternalInput")
    d_w = nc.dram_tensor("wpack", [128, WPACK_COLS], dt.bfloat16, kind="ExternalInput")
    d_kwb = nc.dram_tensor("kwb", [65, 128], dt.bfloat16, kind="ExternalInput")
    d_bias = nc.dram_tensor("biases", [128, NB], dt.float32, kind="ExternalInput")
    d_oh = nc.dram_tensor("onehot", [4, KH * 128], dt.bfloat16, kind="ExternalInput")
    d_outv = nc.dram_tensor("out_v", [128, NC], dt.float32, kind="ExternalOutput")
    d_outs = nc.dram_tensor("out_s", [128, GC], dt.float32, kind="ExternalOutput")

    with tile.TileContext(nc) as tc:
        with (
            tc.tile_pool(name="const", bufs=1) as cpool,
            tc.tile_pool(name="pers", bufs=1) as ppool,
            tc.tile_pool(name="ewin", bufs=2) as ewin,
            tc.tile_pool(name="chk", bufs=2) as chk,
            tc.tile_pool(name="dn", bufs=2) as dnp,
            tc.tile_pool(name="psw", bufs=2, space="PSUM") as psw,
            tc.tile_pool(name="pske", bufs=2, space="PSUM") as pske,
            tc.tile_pool(name="pssm", bufs=4, space="PSUM") as pssm,
        ):
            # ---------------- constants
            w_sb = cpool.tile([128, WPACK_COLS], dt.bfloat16)
            nc.sync.dma_start(w_sb[:], d_w[:])
            kwb_sb = cpool.tile([65, 128], dt.bfloat16)
            nc.sync.dma_start(kwb_sb[:], d_kwb[:])
            bias_sb = cpool.tile([128, NB], dt.float32)
            nc.sync.dma_start(bias_sb[:], d_bias[:])
            vfm_sb = ppool.tile([128, NC], dt.bfloat16)
            nc.sync.dma_start(vfm_sb[:], d_vfm[:])
            sfm_sb = cpool.tile([128, GC], dt.bfloat16)
            nc.sync.dma_start(sfm_sb[:], d_sfm[:])

            def W(name, j0=0, w=128):
                o = COLMAP[name]
                return w_sb[:, o + j0: o + j0 + w]

            def B(name):
                return bias_sb[:, BMAP[name]:BMAP[name] + 1]

            # ---------------- persistent intermediates
            sve_sb = ppool.tile([128, NC], dt.bfloat16)
            aexp_sb = ppool.tile([4, GC, RG], dt.bfloat16)
            pooled_sb = ppool.tile([128, KH, GC], dt.float32)
            pooled_bf = ppool.tile([128, KH * GC], dt.bfloat16)
            s2m_sb = cpool.tile([128, GC], dt.bfloat16)
            denom_sb = cpool.tile([4, GC], dt.float32)
            rden_sb = cpool.tile([4, GC], dt.bfloat16)

            # s2m = tanh(C^T s + C_b)
            ps = pssm.tile([128, CHUNK], dt.float32, tag="ps500")
            nc.tensor.matmul(ps[:, :GC], W("C"), sfm_sb[:], start=True, stop=True)
            nc.scalar.activation(s2m_sb[:], ps[:, :GC], Act.Tanh, bias=B("C_b"))

            # ---------------- P1: edge pipeline (gather segments x windows)
            pw = None
            cur_w = -1
            for si in range(NSEG):
                t0, t1 = si * SEGT, min((si + 1) * SEGT, T_TOT)
                nt = t1 - t0
                e_sb = ewin.tile([65, nt * 128], dt.bfloat16, tag="e_sb")
                nc.sync.dma_start(e_sb[:], d_efm[:, t0 * 128: t1 * 128])
                s_sb = ewin.tile([128, nt * SW], dt.bfloat16, tag="s_sb")
                nc.sync.dma_start(s_sb[:], d_S[:, t0 * SW: t1 * SW])
                vsrc = ewin.tile([128, nt, 128], dt.bfloat16, tag="vsrc")
                nc.sync.dma_start(
                    vsrc[:].rearrange("p a b -> p (a b)"),
                    d_vsrc[:, t0 * VD: t1 * VD])
                ve = ewin.tile([128, nt, 128], dt.bfloat16, tag="ve")
                for s0 in range(0, nt, SUB):
                    s1 = min(s0 + SUB, nt)
                    kep = pske.tile([128, SUB * 128], dt.float32, tag="kep")
                    for t in range(s0, s1):
                        nc.tensor.matmul(
                            kep[:, (t - s0) * 128: (t - s0 + 1) * 128],
                            e_sb[:, t * 128: (t + 1) * 128],
                            kwb_sb[:], start=True, stop=True)
                    ncols = (s1 - s0) * 128
                    vep = ewin.tile([128, SUB * 128], dt.bfloat16, tag="vep")
                    nc.vector.tensor_tensor(
                        vep[:, :ncols], kep[:, :ncols],
                        vsrc[:, s0:s1, :].rearrange("p a b -> p (a b)"),
                        op=Alu.mult)
                    nc.scalar.activation(
                        ve[:, s0:s1, :].rearrange("p a b -> p (a b)"),
                        vep[:, :ncols], Act.Prelu, alpha=0.1)
                # scatter-accumulate, handling window boundaries
                for t in range(t0, t1):
                    wi = int(TWIN[t])
                    if wi != cur_w:
                        if cur_w >= 0:
                            nc.vector.tensor_copy(
                                sve_sb[:, WIN_STARTS[cur_w]:
                                       WIN_STARTS[cur_w] + WIN_SIZES[cur_w]],
                                pw[:, :WIN_SIZES[cur_w]])
                        pw = psw.tile([128, WIN], dt.float32, tag="pw")
                        nc.vector.memset(pw[:], 0.0)
                        cur_w = wi
                    b = int(BASE[t])
                    nc.tensor.matmul(
                        pw[:, b: b + SW], ve[:, t - t0, :],
                        s_sb[:, (t - t0) * SW: (t - t0 + 1) * SW],
                        start=False, stop=(t == T_TOT - 1 or TWIN[t + 1] != wi),
                        skip_group_check=True)
            nc.vector.tensor_copy(
                sve_sb[:, WIN_STARTS[cur_w]: WIN_STARTS[cur_w] + WIN_SIZES[cur_w]],
                pw[:, :WIN_SIZES[cur_w]])

            # ---------------- P2 prologue: d_super, Wmat (padded per head)
            wmatp = []
            for k in range(KH):
                dsp = pssm.tile([128, CHUNK], dt.float32, tag="ps500")
                nc.tensor.matmul(dsp[:, :GC], W("mB", k * 128), sfm_sb[:],
                                 start=True, stop=True)
                ds_k = cpool.tile([128, GC], dt.bfloat16, tag=f"ds{k}")
                nc.scalar.activation(ds_k[:], dsp[:, :GC], Act.Tanh, bias=B(f"mB_b{k}"))
                wp = cpool.tile([128, GC, KH], dt.bfloat16, tag=f"wmatp{k}")
                nc.vector.memset(wp[:], 0.0)
                nc.vector.tensor_scalar(wp[:, :, k], ds_k[:], B(f"mC_{k}"), None,
                                        op0=Alu.mult)
                wmatp.append(wp)

            # ---------------- P2: attention
            for ci in range(NCH):
                nsl = slice(ci * CHUNK, (ci + 1) * CHUNK)
                dns = []
                for k in range(KH):
                    dnps = pssm.tile([128, CHUNK], dt.float32, tag="ps500")
                    nc.tensor.matmul(dnps[:], W("mA", k * 128), vfm_sb[:, nsl],
                                     start=True, stop=True)
                    dn_k = dnp.tile([128, CHUNK], dt.bfloat16, tag=f"dn{k}")
                    nc.scalar.activation(dn_k[:], dnps[:], Act.Tanh,
                                         bias=B(f"mA_b{k}"))
                    dns.append(dn_k)
                aps = pssm.tile([4, CHUNK], dt.float32, tag="ps500")
                for gl in range(GPC):
                    g = ci * GPC + gl
                    for k in range(KH):
                        nc.tensor.matmul(
                            aps[:, gl * RG: (gl + 1) * RG],
                            wmatp[k][:, g, :],
                            dns[k][:, gl * RG: (gl + 1) * RG],
                            start=(k == 0), stop=(k == KH - 1))
                nc.scalar.activation(
                    aexp_sb[:, ci * GPC:(ci + 1) * GPC, :],
                    aps[:].rearrange("p (a b) -> p a b", a=GPC), Act.Exp)

            nc.vector.tensor_reduce(denom_sb[:], aexp_sb[:], axis=Axis.X, op=Alu.add)
            with nc.allow_low_precision(reason="bf16 recip ok at 2e-2 gate"):
                nc.vector.reciprocal(rden_sb[:], denom_sb[:])

            # one-hot rows for PE partition-broadcast (row k of a 4-row
            # tensor -> all 128 partitions): oh[c, 128k+h] = (c == k)
            oh_sb = cpool.tile([4, KH * 128], dt.bfloat16)
            nc.sync.dma_start(oh_sb[:], d_oh[:])

            for ci in range(NCH):
                nsl = slice(ci * CHUNK, (ci + 1) * CHUNK)
                for k in range(KH):
                    abp = pssm.tile([128, CHUNK], dt.float32, tag="ps500")
                    nc.tensor.matmul(
                        abp[:].rearrange("p (a b) -> p a b", a=GPC),
                        oh_sb[:, k * 128:(k + 1) * 128],
                        aexp_sb[0:4, ci * GPC:(ci + 1) * GPC, :],
                        start=True, stop=True)
                    ab = chk.tile([128, CHUNK], dt.bfloat16, tag="ab")
                    nc.scalar.activation(ab[:], abp[:], Act.Copy)
                    dvp = pssm.tile([128, CHUNK], dt.float32, tag="ps500")
                    nc.tensor.matmul(dvp[:], W("mD", k * 128), vfm_sb[:, nsl],
                                     start=True, stop=True)
                    dvw = chk.tile([128, GPC, RG], dt.bfloat16, tag="dvw")
                    nc.vector.tensor_tensor(
                        dvw[:].rearrange("p a b -> p (a b)"), dvp[:], ab[:],
                        op=Alu.mult)
                    nc.vector.tensor_reduce(
                        pooled_sb[:, k, ci * GPC:(ci + 1) * GPC], dvw[:],
                        axis=Axis.X, op=Alu.add)
            # pooled_bf = pooled_raw * rden (broadcast rden rows via PE)
            rbp = pssm.tile([128, CHUNK], dt.float32, tag="ps500")
            for k in range(KH):
                nc.tensor.matmul(rbp[:, k * GC:(k + 1) * GC],
                                 oh_sb[:, k * 128:(k + 1) * 128],
                                 rden_sb[0:4, :], start=True, stop=True)
            rden_bc = chk.tile([128, KH * GC], dt.bfloat16, tag="rdenbc")
            nc.scalar.activation(rden_bc[:], rbp[:], Act.Copy)
            nc.vector.tensor_tensor(pooled_bf[:],
                                    pooled_sb[:].rearrange("p a b -> p (a b)"),
                                    rden_bc[:], op=Alu.mult)

            # ---------------- P3: message + GRU per node chunk
            for ci in range(NCH):
                nsl = slice(ci * CHUNK, (ci + 1) * CHUNK)
                gsl = slice(ci * GPC, (ci + 1) * GPC)
                s2m_b3 = s2m_sb[:, gsl].unsqueeze(2).to_broadcast([128, GPC, RG])

                def r3(ap):
                    return ap.rearrange("p (a b) -> p a b", a=GPC)

                mp = pssm.tile([128, CHUNK], dt.float32, tag="ps500")
                nc.tensor.matmul(mp[:], W("E", 0), sve_sb[:, nsl], start=True, stop=False)
                nc.tensor.matmul(mp[:], W("E", 128), vfm_sb[:, nsl], start=False, stop=True)
                m2m = chk.tile([128, CHUNK], dt.bfloat16, tag="m2m")
                nc.scalar.activation(m2m[:], mp[:], Act.Prelu, bias=B("E_b"), alpha=0.1)

                zp = pssm.tile([128, CHUNK], dt.float32, tag="ps500")
                nc.tensor.matmul(zp[:], W("gmA"), m2m[:], start=True, stop=False)
                nc.tensor.matmul(zp[:].rearrange("p (a b) -> p a b", a=GPC),
                                 W("gmB"), s2m_b3, start=False, stop=True)
                zt = chk.tile([128, CHUNK], dt.bfloat16, tag="zt")
                nc.scalar.activation(zt[:], zp[:], Act.Sigmoid, bias=B("gmz"))

                t1 = chk.tile([128, CHUNK], dt.bfloat16, tag="t1")
                nc.vector.tensor_tensor(r3(t1[:]), s2m_b3, r3(m2m[:]), op=Alu.subtract)
                t2 = chk.tile([128, CHUNK], dt.bfloat16, tag="t2")
                nc.vector.tensor_tensor(t2[:], zt[:], t1[:], op=Alu.mult)
                hm = chk.tile([128, CHUNK], dt.bfloat16, tag="hm")
                nc.vector.tensor_tensor(hm[:], t2[:], m2m[:], op=Alu.add)

                rp = pssm.tile([128, CHUNK], dt.float32, tag="ps500")
                nc.tensor.matmul(rp[:], W("gmWih", 0), vfm_sb[:, nsl], start=True, stop=False)
                nc.tensor.matmul(rp[:], W("gmWhh", 0), hm[:], start=False, stop=True)
                r = chk.tile([128, CHUNK], dt.bfloat16, tag="r")
                nc.scalar.activation(r[:], rp[:], Act.Sigmoid, bias=B("gm_br"))

                z2p = pssm.tile([128, CHUNK], dt.float32, tag="ps500")
                nc.tensor.matmul(z2p[:], W("gmWih", 128), vfm_sb[:, nsl], start=True, stop=False)
                nc.tensor.matmul(z2p[:], W("gmWhh", 128), hm[:], start=False, stop=True)
                z2 = chk.tile([128, CHUNK], dt.bfloat16, tag="z2")
                nc.scalar.activation(z2[:], z2p[:], Act.Sigmoid, bias=B("gm_bz"))

                innp = pssm.tile([128, CHUNK], dt.float32, tag="ps500")
                nc.tensor.matmul(innp[:], W("gmWih", 256), vfm_sb[:, nsl], start=True, stop=True)
                hnp = pssm.tile([128, CHUNK], dt.float32, tag="ps500")
                nc.tensor.matmul(hnp[:], W("gmWhh", 256), hm[:], start=True, stop=True)
                t3 = chk.tile([128, CHUNK], dt.float32, tag="t3")
                nc.vector.scalar_tensor_tensor(t3[:], hnp[:], B("gm_bhhn"), r[:],
                                               op0=Alu.add, op1=Alu.mult)
                t4 = chk.tile([128, CHUNK], dt.float32, tag="t4")
                nc.vector.tensor_tensor(t4[:], t3[:], innp[:], op=Alu.add)
                n = chk.tile([128, CHUNK], dt.bfloat16, tag="n")
                nc.scalar.activation(n[:], t4[:], Act.Tanh, bias=B("gm_bihn"))
                u1 = chk.tile([128, CHUNK], dt.bfloat16, tag="u1")
                nc.vector.tensor_tensor(u1[:], hm[:], n[:], op=Alu.subtract)
                u2 = chk.tile([128, CHUNK], dt.bfloat16, tag="u2")
                nc.vector.tensor_tensor(u2[:], z2[:], u1[:], op=Alu.mult)
                uo = chk.tile([128, CHUNK], dt.float32, tag="uo")
                nc.vector.tensor_tensor(uo[:], u2[:], n[:], op=Alu.add)
                nc.sync.dma_start(d_outv[:, nsl], uo[:])

            # ---------------- P4: supernode side (125 graphs at once)
            def gmm(w1, r1, w2=None, r2=None):
                p = pssm.tile([128, CHUNK], dt.float32, tag="ps500")
                nc.tensor.matmul(p[:, :GC], w1, r1, start=True, stop=(w2 is None))
                if w2 is not None:
                    nc.tensor.matmul(p[:, :GC], w2, r2, start=False, stop=True)
                return p

            def act(p, func, bias, tag):
                o = chk.tile([128, GC], dt.bfloat16, tag=tag)
                nc.scalar.activation(o[:], p[:, :GC], func, bias=B(bias))
                return o

            s2s = act(gmm(W("A"), sfm_sb[:]), Act.Tanh, "A_b", "s2s")
            p = pssm.tile([128, CHUNK], dt.float32, tag="ps500")
            for k in range(KH):
                nc.tensor.matmul(p[:, :GC], W("B", k * 128),
                                 pooled_bf[:, k * GC:(k + 1) * GC],
                                 start=(k == 0), stop=(k == KH - 1))
            m2s = chk.tile([128, GC], dt.bfloat16, tag="m2s")
            nc.scalar.activation(m2s[:], p[:, :GC], Act.Tanh, bias=B("B_b"))
            zs = act(gmm(W("gsA"), s2s[:], W("gsB"), m2s[:]), Act.Sigmoid, "gsz", "zs")
            st1 = chk.tile([128, GC], dt.bfloat16, tag="st1")
            nc.vector.tensor_tensor(st1[:], m2s[:], s2s[:], op=Alu.subtract)
            st2 = chk.tile([128, GC], dt.bfloat16, tag="st2")
            nc.vector.tensor_tensor(st2[:], zs[:], st1[:], op=Alu.mult)
            hs = chk.tile([128, GC], dt.bfloat16, tag="hs")
            nc.vector.tensor_tensor(hs[:], st2[:], s2s[:], op=Alu.add)
            rs = act(gmm(W("gsWih", 0), sfm_sb[:], W("gsWhh", 0), hs[:]),
                     Act.Sigmoid, "gs_br", "rs")
            z2s = act(gmm(W("gsWih", 128), sfm_sb[:], W("gsWhh", 128), hs[:]),
                      Act.Sigmoid, "gs_bz", "z2s")
            innp = gmm(W("gsWih", 256), sfm_sb[:])
            hnp = gmm(W("gsWhh", 256), hs[:])
            st3 = chk.tile([128, GC], dt.float32, tag="st3")
            nc.vector.scalar_tensor_tensor(st3[:], hnp[:, :GC], B("gs_bhhn"), rs[:],
                                           op0=Alu.add, op1=Alu.mult)
            st4 = chk.tile([128, GC], dt.float32, tag="st4")
            nc.vector.tensor_tensor(st4[:], st3[:], innp[:, :GC], op=Alu.add)
            ns = chk.tile([128, GC], dt.bfloat16, tag="ns")
            nc.scalar.activation(ns[:], st4[:], Act.Tanh, bias=B("gs_bihn"))
            su1 = chk.tile([128, GC], dt.bfloat16, tag="su1")
            nc.vector.tensor_tensor(su1[:], hs[:], ns[:], op=Alu.subtract)
            su2 = chk.tile([128, GC], dt.bfloat16, tag="su2")
            nc.vector.tensor_tensor(su2[:], z2s[:], su1[:], op=Alu.mult)
            suo = chk.tile([128, GC], dt.float32, tag="suo")
            nc.vector.tensor_tensor(suo[:], su2[:], ns[:], op=Alu.add)
            nc.sync.dma_start(d_outs[:], suo[:])

    nc.compile()
    return nc


# ===========================================================================
# Entry point
# ===========================================================================

def prepare(inputs):
    meta, slot_eid = _pack_edges(inputs["edge_src"], inputs["edge_dst"])
    DMAX = _compute_dmax(meta, slot_eid,
                         np.asarray(inputs["edge_src"]).astype(np.int64))
    wpack, colmap, biases, bmap, kwb = _stage_weights(inputs)
    meta["WPACK_COLS"] = wpack.shape[1]
    meta["COLMAP"] = colmap
    meta["BMAP"] = bmap
    meta["NB"] = biases.shape[1]

    v = np.asarray(inputs["v"], np.float32)
    s = np.asarray(inputs["s"], np.float32)
    onehot = np.zeros((4, KH * 128), np.float32)
    for k in range(KH):
        onehot[k, k * 128:(k + 1) * 128] = 1.0
    onehot = onehot.astype(BF16)
    in_maps = []
    for c in range(NCORES):
        st = _stage_core(c, meta, slot_eid[c], inputs, DMAX)
        in_maps.append(dict(
            vsrc=np.ascontiguousarray(st["vsrc"]),
            e_fm=np.ascontiguousarray(st["e_fm"]),
            S=np.ascontiguousarray(st["S"]),
            v_fm=np.ascontiguousarray(v[c * NC:(c + 1) * NC].T).astype(BF16),
            s_fm=np.ascontiguousarray(s[c * GC:(c + 1) * GC].T).astype(BF16),
            wpack=wpack, kwb=kwb, biases=biases, onehot=onehot,
        ))
    return meta, DMAX, in_maps


def kernel(**inputs):
    meta, DMAX, in_maps = prepare(inputs)
    nc = build_program(meta, DMAX)

    from concourse.bass_utils import run_bass_kernel_spmd
    res = run_bass_kernel_spmd(nc, in_maps, core_ids=list(range(NCORES)))
    upd_v = np.concatenate(
        [np.asarray(res.results[c]["out_v"]).T for c in range(NCORES)], axis=0)
    upd_s = np.concatenate(
        [np.asarray(res.results[c]["out_s"]).T for c in range(NCORES)], axis=0)
    return upd_v.astype(np.float32), upd_s.astype(np.float32)
